# revision 57
# baseline (speedup 1.0000x reference)
"""CapsuleNet forward kernel for 8 Trainium2 NeuronCores.

Data-parallel over batch (64 images / core); the routing b_ij batch-mean
uses an AllReduce per iteration.  u_hat is never materialized: s_j and the
agreement mean are computed directly against W from the 9216-dim flattened
capsule vector u.

Per-core pipeline:
  conv1  : K=81 matmuls with garbage-cropped 200-col rhs (im2col built by
           strided DMA from DRAM, 2240B segments); 8 matmul slots per
           4-bank PSUM tile; one fused ReLU+bias copy per 4 images,
           alternating DVE/Act engines
  conv2  : 324 accumulating K=128 matmuls (81 taps x 2 ci chunks) per co
           chunk over the full local batch (5 image-aligned PSUM banks);
           bias-add drains to bf16 and scatter-writes the capsule layout
           to DRAM (72B segments)
  capsule: u2T[p = b + 64*mc, co*36+s] bf16 [128, 4608]; squash over
           8-elem groups (f32 norms, bf16 scale); u2R = xbar DMA-transpose
           (f-major, cols (h, j, b))
  routing: s_j^T = (c-scaled W)^T @ u2, 72 K-tile accumulation;
           agreement mean m = sum_{o,i} W .* (v2^T @ u2) via rank-64
           matmul + DVE mult/group-reduce + selector matmuls;
           AllReduce(m) -> b_ij update -> softmax.
"""

import numpy as np
import ml_dtypes

import concourse.bacc as bacc
import concourse.bass as bass
import concourse.mybir as mybir
import concourse.tile as tile
from concourse.bass_utils import run_bass_kernel_spmd

F32 = mybir.dt.float32
BF16 = mybir.dt.bfloat16
MUL = mybir.AluOpType.mult
ADD = mybir.AluOpType.add
MAX = mybir.AluOpType.max
AXX = mybir.AxisListType.X
ACT = mybir.ActivationFunctionType

NCORES = 8
B = 512
BL = B // NCORES        # 64 images per core
SB = 16                 # conv1 im2col sub-batch
NSB = BL // SB
J = 560                 # 20 rows x 28 cols (8 garbage cols/row)
JC = 400                # compact 20x20 conv1 output per image
R, C, O, I = 1152, 10, 16, 8
F = R * I               # 9216
FH = F // 2             # 4608 per fold half
CO = C * O              # 160
KT = F // 128           # 72
S2 = 36                 # 6x6 conv2 positions per image
N2 = BL * S2
BCH = [(0, 14), (14, 14), (28, 14), (42, 14), (56, 8)]
NIT = 3


def _sub(ap, off, dims):
    """Arbitrary strided view (offset in elements, dims=[[step,count],..])."""
    return bass.AP(ap.tensor, ap.offset + off, [list(d) for d in dims])


def _pp(ap):
    """Partition pitch (elements per partition row) of an SBUF AP."""
    return ap.ap[0][0]


def build_nc(for_sim=False, reps=1):
    nc = bacc.Bacc("TRN2", target_bir_lowering=False, debug=False,
                   num_devices=1 if for_sim else NCORES)
    nc._for_sim = for_sim

    xin = nc.dram_tensor("xin", [BL * 784 + 8], BF16, kind="ExternalInput").ap()
    cpb = nc.dram_tensor("cpb", [128, 528], BF16, kind="ExternalInput").ap()
    cpf = nc.dram_tensor("cpf", [128, 96], F32, kind="ExternalInput").ap()
    w2s = nc.dram_tensor("w2s", [162, 128, 256], BF16, kind="ExternalInput").ap()
    wlb = nc.dram_tensor("wlb", [128, KT * CO], BF16, kind="ExternalInput").ap()
    out = nc.dram_tensor("out", [BL, CO], F32, kind="ExternalOutput").ap()

    cc_in = nc.dram_tensor("cc_in", [16, KT * C], BF16)
    cc_out = nc.dram_tensor("cc_out", [16, KT * C], BF16,
                            addr_space="Local" if for_sim else "Shared")
    vd = nc.dram_tensor("vd", [128, FH], BF16)         # conv2 out bounce

    with tile.TileContext(nc, num_cores=NCORES) as tc:
        for _rep in range(reps):
            _body(tc, nc, xin, cpb, cpf, w2s, wlb, out, cc_in, cc_out, vd)
    nc.compile()
    return nc


def _body(tc, nc, xin, cpb, cpf, w2s, wlb, out, cc_in, cc_out, vd):
    with tc.tile_pool(name="const", bufs=1) as pc, \
         tc.tile_pool(name="upers", bufs=1) as pU:

        cpb_sb = pc.tile([128, 528], BF16, tag="cpb")
        nc.gpsimd.dma_start(cpb_sb[:], cpb)
        cpf_sb = pc.tile([128, 96], F32, tag="cpf")
        nc.gpsimd.dma_start(cpf_sb[:], cpf)
        w1t_sb = cpb_sb[0:81, 0:256]
        sel8x_sb = cpb_sb[:, 256:272]
        selc_sb = cpb_sb[0:16, 272:400]
        eyeb2_sb = cpb_sb[0:64, 400:528]
        b1_sb = cpf_sb[:, 0:2]
        b2_sb = cpf_sb[:, 2:4]
        ones16_sb = cpf_sb[0:16, 4:5]
        ones1_sb = cpf_sb[0:1, 5:69]
        u2T = pU.tile([128, FH], BF16, tag="u2T")       # folded capsules
        u2R = pU.tile([128, FH], BF16, tag="u2R")       # f-major (h, j, b)
        wsb = pU.tile([128, KT * CO], BF16, tag="wsb")

        # ============ Phase A: conv1 + conv2 + capsule formation ===========
        with tc.tile_pool(name="pH", bufs=1) as pH:
            h1 = [pH.tile([128, BL * JC], BF16, tag=f"h1_{kc}",
                          name=f"h1_{kc}") for kc in range(2)]

            # conv1: 8 matmul slots (4 images x 2 half-rows) per 4-bank
            # PSUM tile; one fused bias+ReLU copy per tile
            with tc.tile_pool(name="pA", bufs=2) as pA, \
                 tc.tile_pool(name="ps1", bufs=4, space="PSUM") as ps1:
                flip = 0
                for half in range(NSB):
                    b0h = half * SB
                    A = pA.tile([81, SB * J], BF16, tag="A")
                    pa = _pp(A[:])
                    # im2col: one DMA per kernel row, alternating the
                    # HWDGE (sync) and SWDGE (gpsimd) issue paths
                    for kh in range(9):
                        eng = nc.sync if kh % 2 == 0 else nc.gpsimd
                        eng.dma_start(
                            _sub(A[:], 9 * kh * pa, [[pa, 9], [J, SB], [1, J]]),
                            _sub(xin, b0h * 784 + 28 * kh,
                                 [[1, 9], [784, SB], [1, J]]))

                    for bg in range(SB // 2):
                        for mc in range(2):
                            lhsT = w1t_sb[:, mc * 128:(mc + 1) * 128]
                            ps = ps1.tile([128, 1024], F32, tag="c1ps")
                            for sl in range(4):      # slot = (img, hf)
                                bi = bg * 2 + sl // 2
                                hf = sl % 2
                                rhs = _sub(A[:], bi * J + hf * 280,
                                           [[pa, 81], [28, 10], [1, 20]])
                                nc.tensor.matmul(
                                    ps[:, sl * 256: sl * 256 + 200],
                                    lhsT, rhs, start=True, stop=True)
                            doff = (b0h + bg * 2) * JC
                            dstc = _sub(h1[mc][:], doff,
                                        [[_pp(h1[mc][:]), 128], [200, 4],
                                         [1, 200]])
                            srcc = _sub(ps[:], 0,
                                        [[_pp(ps[:]), 128], [256, 4], [1, 200]])
                            bb = b1_sb[:, mc:mc + 1]
                            if flip % 2 == 0:
                                nc.vector.tensor_scalar(dstc, srcc, bb, 0.0,
                                                        op0=ADD, op1=MAX)
                            else:
                                nc.scalar.activation(dstc, srcc, ACT.Relu,
                                                     bias=bb)
                            flip += 1

            # routing weights: contiguous loads on the Act DMA queue,
            # floored past conv1's im2col so they don't steal DMA bandwidth
            with tc.tile_wait_until(0.018):
                nc.gpsimd.dma_start(wsb[:], wlb)

            # conv2 (the mc=0 half's squash hides under mc=1's matmuls)
            with tc.tile_pool(name="pW2", bufs=12) as pW2, \
                 tc.tile_pool(name="pV", bufs=3) as pV, \
                 tc.tile_pool(name="squ", bufs=3) as pq0, \
                 tc.tile_pool(name="ps2", bufs=1, space="PSUM") as ps2:
                NGC = 1152 // I          # squash chunk: 1152 f, 144 groups

                def squash_half(h):
                    # squash u2T rows [64h, 64h+64) in 4 column chunks,
                    # then DMA-transpose the half into u2R
                    rows = slice(h * 64, h * 64 + 64)
                    for qc in range(4):
                        fsl = slice(qc * 1152, (qc + 1) * 1152)
                        uv = u2T[rows, fsl]
                        sqr = pq0.tile([128, 1152], F32, tag="sqr", name="sqr")[rows, :]
                        nc.scalar.square(sqr, uv)
                        sq = pq0.tile([128, NGC], F32, tag="sq", name="sq")[rows, :]
                        nc.vector.tensor_reduce(
                            sq, sqr.rearrange("p (r i) -> p r i", i=I),
                            axis=AXX, op=ADD)
                        srt = pq0.tile([128, NGC], F32, tag="srt", name="srt")[rows, :]
                        nc.scalar.sqrt(srt, sq)
                        d2 = pq0.tile([128, NGC], F32, tag="d2", name="d2")[rows, :]
                        nc.vector.scalar_tensor_tensor(d2, sq, 1.0, srt,
                                                       op0=ADD, op1=MUL)
                        rc = pq0.tile([128, NGC], F32, tag="rc", name="rc")[rows, :]
                        nc.vector.reciprocal(rc, d2)
                        g = pq0.tile([128, NGC], F32, tag="g", name="g")[rows, :]
                        nc.vector.tensor_mul(g, sq, rc)
                        gx = pq0.tile([128, 1152], BF16, tag="gx", name="gx")[rows, :]
                        ppg = _pp(g)
                        ppx = _pp(gx)
                        nc.scalar.activation(
                            _sub(gx, 0, [[ppx, 64], [I, NGC], [1, I]]),
                            _sub(g, 0, [[ppg, 64], [1, NGC], [0, I]]),
                            ACT.Copy)
                        nc.vector.tensor_mul(uv, uv, gx)
                        # u2R[q, h*2304+j*64+b] = u2T[b+64h, j*128+q];
                        # per-chunk so the last chunk gates less
                        nc.sync.dma_start_transpose(
                            _sub(u2R[:], h * 2304 + qc * 576,
                                 [[_pp(u2R[:]), 128], [64, 9], [1, 64]]),
                            u2T[rows, fsl])

                for mc in range(2):
                    pss = [ps2.tile([128, nb * S2], F32, tag=f"c2ps{i}",
                                    name=f"c2ps{i}_{mc}")
                           for i, (_, nb) in enumerate(BCH)]
                    for khw in range(81):
                        kh2, kw2 = khw // 9, khw % 9
                        wch = pW2.tile([128, 512], BF16, tag="wch")
                        nc.sync.dma_start(
                            _sub(wch[:], 0,
                                 [[_pp(wch[:]), 128], [256, 2], [1, 256]]),
                            _sub(w2s, khw * 2 * 128 * 256,
                                 [[256, 128], [128 * 256, 2], [1, 256]]))
                        for kc in range(2):
                            lhsT = wch[:, kc * 256 + mc * 128:
                                        kc * 256 + mc * 128 + 128]
                            for ic, (b0, nb) in enumerate(BCH):
                                rhs = _sub(h1[kc][:], b0 * JC + 20 * kh2 + kw2,
                                           [[_pp(h1[kc][:]), 128],
                                            [JC, nb], [40, 6], [2, 6]])
                                nc.tensor.matmul(
                                    pss[ic][:], lhsT, rhs,
                                    start=(kc == 0 and khw == 0),
                                    stop=(kc == 1 and khw == 80))
                    v = pV.tile([128, N2], BF16, tag="v")
                    for ic, (b0, nb) in enumerate(BCH):
                        nc.scalar.activation(v[:, b0 * S2:(b0 + nb) * S2],
                                             pss[ic][:], ACT.Identity,
                                             bias=b2_sb[:, mc:mc + 1])
                        # scatter-write this chunk: vd[b+64mc, co*36+s]
                        nc.sync.dma_start(
                            _sub(vd.ap(), mc * 64 * FH + b0 * FH,
                                 [[36, 128], [FH, nb], [1, 36]]),
                            v[:, b0 * S2:(b0 + nb) * S2])
                    # chunked read back into the folded SBUF tile so the
                    # squash column-chunks start as their data lands
                    for qc in range(4):
                        nc.sync.dma_start(
                            u2T[mc * 64:(mc + 1) * 64,
                                qc * 1152:(qc + 1) * 1152],
                            _sub(vd.ap(), mc * 64 * FH + qc * 1152,
                                 [[FH, 64], [1, 1152]]))
                    squash_half(mc)

        if True:
            # ============ routing ==========================================
            # co-order is (o, c): co' = o*10 + c. m/b_ij/csm live in the
            # [16 r_local, 72 t * 10 c] layout; r = 16t + r_local.
            with tc.tile_pool(name="pB", bufs=1) as pB, \
                 tc.tile_pool(name="pTb", bufs=4) as pTb, \
                 tc.tile_pool(name="pPm", bufs=4) as pPm, \
                 tc.tile_pool(name="psq2", bufs=2) as pq, \
                 tc.tile_pool(name="psB", bufs=2, space="PSUM") as psB, \
                 tc.tile_pool(name="psS", bufs=1, space="PSUM") as psS:

                wp = pB.tile([128, KT * CO], BF16, tag="wp")
                adum = pB.tile([1, 2], F32, tag="adum")
                cE = pB.tile([128, KT * C], BF16, tag="cE")
                csmA = pB.tile([16, KT * C], BF16, tag="csmA")
                csmB = pB.tile([16, KT * C], BF16, tag="csmB")
                csms = [csmA, csmB]
                mAllN = pB.tile([16, KT * C], BF16, tag="mAllN")
                msum = pB.tile([16, KT * C], BF16, tag="msum")
                pro = pB.tile([128, KT * C], BF16, tag="pro")
                v2T = pB.tile([BL, CO], F32, tag="v2T")
                dinv64 = pB.tile([BL, C], F32, tag="dinv64")
                v2Tb = pB.tile([BL, CO], BF16, tag="v2Tb")
                v2rep = pB.tile([128, CO], BF16, tag="v2rep")

                lam = 1.0 / R
                for it in range(NIT):
                    if it > 0:
                        # cE[8*rl+i, (t,c)] = csm[rl, (t,c)] via selector
                        csm16 = csms[it - 1]
                        for hf2 in range(2):
                            ceps = psB.tile([128, 512], F32, tag="ceps",
                                            name="ceps", bufs=2)
                            nc.tensor.matmul(
                                ceps[:, 0:360],
                                selc_sb, csm16[:, hf2 * 360:(hf2 + 1) * 360],
                                start=True, stop=True)
                            nc.scalar.activation(
                                cE[:, hf2 * 360:(hf2 + 1) * 360],
                                ceps[:, 0:360], ACT.Copy)
                        # wp = wsb * broadcast_o(cE), two 2x half-passes so
                        # s_j's first K-tiles can start after half A
                        ppw = _pp(wp[:])
                        pps = _pp(wsb[:])
                        ppe = _pp(cE[:])
                        for wh in range(2):
                            nc.vector.tensor_tensor(
                                _sub(wp[:], wh * 36 * CO,
                                     [[ppw, 128], [CO, 36], [C, O], [1, C]]),
                                _sub(wsb[:], wh * 36 * CO,
                                     [[pps, 128], [CO, 36], [C, O], [1, C]]),
                                _sub(cE[:], wh * 36 * C,
                                     [[ppe, 128], [C, 36], [0, O], [1, C]]),
                                op=MUL)

                    # s_j^T [b, (o,c)] over 72 accumulating K-tiles
                    wcur = wsb if it == 0 else wp
                    ssum = psS.tile([128, 512], F32, tag="sv")
                    for t in range(KT):
                        j, h = t % 36, t // 36
                        lhsT = _sub(u2R[:], h * 2304 + j * 64,
                                    [[_pp(u2R[:]), 128], [1, BL]])
                        nc.tensor.matmul(ssum[0:BL, 0:CO], lhsT,
                                         wcur[:, t * CO:(t + 1) * CO],
                                         start=(t == 0), stop=(t == KT - 1))

                    # v2 = squash(s) over o-groups (iter0 folds 1/R)
                    svr = pq.tile([BL, CO], F32, tag="svr")
                    nc.scalar.square(svr[:], ssum[0:BL, 0:CO])
                    sqv = pq.tile([BL, C], F32, tag="sqv")
                    ppsv = _pp(svr[:])
                    nc.vector.tensor_reduce(
                        sqv[:],
                        _sub(svr[:], 0, [[ppsv, BL], [1, C], [C, O]]),
                        axis=AXX, op=ADD)
                    if it == 0:
                        nc.vector.tensor_scalar(sqv[:], sqv[:], lam * lam,
                                                None, op0=MUL)
                    else:
                        # fold the softmax 1/sum(c) (squared) into |s|^2
                        dq = pq.tile([BL, C], F32, tag="dq")
                        nc.vector.tensor_mul(dq[:], dinv64[:], dinv64[:])
                        nc.vector.tensor_mul(sqv[:], sqv[:], dq[:])
                    srtv = pq.tile([BL, C], F32, tag="srtv")
                    nc.scalar.sqrt(srtv[:], sqv[:])
                    dv2 = pq.tile([BL, C], F32, tag="dv2")
                    nc.vector.scalar_tensor_tensor(dv2[:], sqv[:], 1.0,
                                                   srtv[:], op0=ADD, op1=MUL)
                    rcv = pq.tile([BL, C], F32, tag="rcv")
                    nc.vector.reciprocal(rcv[:], dv2[:])
                    gv = pq.tile([BL, C], F32, tag="gv")
                    nc.vector.tensor_mul(gv[:], sqv[:], rcv[:])
                    if it == 0:
                        nc.vector.tensor_scalar(gv[:], gv[:], lam, None,
                                                op0=MUL)
                    else:
                        nc.vector.tensor_mul(gv[:], gv[:], dinv64[:])
                    ppv = _pp(v2T[:])
                    pps2 = _pp(ssum[:])
                    ppgv = _pp(gv[:])
                    nc.vector.tensor_tensor(
                        _sub(v2T[:], 0, [[ppv, BL], [C, O], [1, C]]),
                        _sub(ssum[:], 0, [[pps2, BL], [C, O], [1, C]]),
                        _sub(gv[:], 0, [[ppgv, BL], [0, O], [1, C]]),
                        op=MUL)

                    if it == NIT - 1:
                        # out stays in (o, c) order; host transposes
                        nc.sync.dma_start(out, v2T[:])
                        break

                    # v replicated to both partition halves via PE selector
                    nc.scalar.activation(v2Tb[:], v2T[:], ACT.Copy)
                    vrp = psS.tile([128, 512], F32, tag="sv")
                    nc.tensor.matmul(vrp[:, 0:CO], eyeb2_sb, v2Tb[:],
                                     start=True, stop=True)
                    nc.scalar.activation(v2rep[:], vrp[:, 0:CO], ACT.Copy)

                    # agreement: T'f[f, (o,c)] = sum_b u[b,f] v[b,(o,c)],
                    # then m16[rl, (t,c)] = sum_{i,o} wsb .* T'f
                    for ggr in range(KT // 6):
                        tfp = psB.tile([128, 1024], F32, tag="tfp")
                        for dt6 in range(6):
                            t = ggr * 6 + dt6
                            j, h = t % 36, t // 36
                            off = (dt6 // 3) * 512 + (dt6 % 3) * CO
                            lhsT = u2T[h * 64:(h + 1) * 64,
                                       j * 128:(j + 1) * 128]
                            nc.tensor.matmul(tfp[:, off:off + CO],
                                             lhsT,
                                             v2rep[h * 64:(h + 1) * 64, :],
                                             start=True, stop=True)
                        tpb = pTb.tile([128, 960], BF16, tag="tpb")
                        ppt = _pp(tpb[:])
                        ppf = _pp(tfp[:])
                        nc.scalar.activation(
                            _sub(tpb[:], 0, [[ppt, 128], [480, 2], [1, 480]]),
                            _sub(tfp[:], 0, [[ppf, 128], [512, 2], [1, 480]]),
                            ACT.Copy)
                        pm = pPm.tile([128, 960], BF16, tag="pm")
                        mule = nc.vector if ggr % 2 == 0 else nc.gpsimd
                        mule.tensor_tensor(
                            pm[:], tpb[:], wsb[:, ggr * 960:(ggr + 1) * 960],
                            op=MUL)
                        # pre-add the o-halves (bf16 2x), then grouped reduce
                        ph = pPm.tile([128, 480], BF16, tag="ph")
                        ppm = _pp(pm[:])
                        pph = _pp(ph[:])
                        with nc.allow_low_precision(reason="m16 in bf16 ok"):
                            nc.vector.tensor_tensor(
                                _sub(ph[:], 0, [[pph, 128], [80, 6], [1, 80]]),
                                _sub(pm[:], 0, [[ppm, 128], [CO, 6], [1, 80]]),
                                _sub(pm[:], 80, [[ppm, 128], [CO, 6], [1, 80]]),
                                op=ADD)
                            ppr = _pp(pro[:])
                            nc.vector.tensor_reduce(
                                _sub(pro[:], ggr * 60,
                                     [[ppr, 128], [10, 6], [1, 10]]),
                                _sub(ph[:], 0,
                                     [[pph, 128], [80, 6], [1, C], [C, 8]]),
                                axis=AXX, op=ADD)
                    # i-sum via selector: m16[rl, (t,c)]
                    for hf2 in range(2):
                        m16p = psB.tile([16, 512], F32, tag="m16p",
                                        name="m16p", bufs=1)
                        nc.tensor.matmul(
                            m16p[:, 0:360],
                            sel8x_sb, pro[:, hf2 * 360:(hf2 + 1) * 360],
                            start=True, stop=True)
                        nc.scalar.activation(
                            mAllN[:, hf2 * 360:(hf2 + 1) * 360],
                            m16p[:, 0:360], ACT.Copy)
                    nc.scalar.activation(adum[:, 0:1], adum[:, 1:2], ACT.Exp)
                    nc.sync.dma_start(cc_in.ap(), mAllN[:])
                    if getattr(nc, "_for_sim", False):
                        nc.sync.dma_start(cc_out.ap(), cc_in.ap())
                    else:
                        nc.gpsimd.collective_compute(
                            "AllReduce", ADD,
                            replica_groups=[list(range(NCORES))],
                            ins=[cc_in.ap()], outs=[cc_out.ap()])
                    nc.sync.dma_start(msum[:], cc_out.ap())
                    # softmax over routes, UNNORMALIZED and INCREMENTAL:
                    # csm_k = exp(sum of m/B updates) = exp(msum/B) * csm_{k-1};
                    # the 1/sum(c) normalization is folded into the next
                    # v-squash scalars (s_j is linear in c). No max-subtract
                    # (|b_ij| stays O(1), far from the exp overflow range).
                    cs = csms[it]
                    if it == 0:
                        nc.scalar.activation(cs[:], msum[:], ACT.Exp,
                                             scale=1.0 / B)
                    else:
                        expm = pq.tile([16, KT * C], BF16, tag="expm")
                        nc.scalar.activation(expm[:], msum[:], ACT.Exp,
                                             scale=1.0 / B)
                        nc.vector.tensor_mul(cs[:], expm[:], csmA[:])
                    tsum = pq.tile([16, C], F32, tag="tsum")
                    ppb = _pp(cs[:])
                    nc.vector.tensor_reduce(
                        tsum[:],
                        _sub(cs[:], 0, [[ppb, 16], [1, C], [C, KT]]),
                        axis=AXX, op=ADD)
                    # partition-sum (16 -> 1), recip, broadcast to 64 b-rows
                    dsp = psS.tile([128, 512], F32, tag="sv", name="dspv")[0:64, 256:384]
                    nc.tensor.matmul(dsp[0:1, 0:C], ones16_sb, tsum[:],
                                     start=True, stop=True)
                    rcp = pq.tile([1, C], F32, tag="rcp")
                    nc.vector.reciprocal(rcp[:], dsp[0:1, 0:C])
                    nc.tensor.matmul(dsp[0:64, 64:64 + C], ones1_sb,
                                     rcp[:], start=True, stop=True)
                    nc.vector.tensor_copy(dinv64[:], dsp[0:64, 64:64 + C])
                    nc.scalar.sqrt(adum[:, 0:1], adum[:, 1:2])


# ------------------------- host side ---------------------------------------
_CACHE = {}


def make_in_maps(x, conv1_w, conv1_b, conv2_w, conv2_b, W):
    bf = ml_dtypes.bfloat16
    xf = np.ascontiguousarray(np.asarray(x, np.float32).reshape(B, 784))
    w1 = np.ascontiguousarray(
        np.asarray(conv1_w, np.float32).reshape(256, 81).T).astype(bf)
    b1v = np.asarray(conv1_b, np.float32).reshape(2, 128).T.copy()
    w2 = np.asarray(conv2_w, np.float32).reshape(256, 256, 81)
    w2 = np.ascontiguousarray(w2.transpose(2, 1, 0)).reshape(162, 128, 256).astype(bf)
    b2v = np.asarray(conv2_b, np.float32).reshape(2, 128).T.copy()
    cpb = np.zeros((128, 528), bf)
    cpf = np.zeros((128, 96), np.float32)
    cpb[0:81, 0:256] = w1
    cpf[:, 0:2] = b1v
    cpf[:, 2:4] = b2v
    Wf = np.asarray(W, np.float32)
    # wlb rows q = f%128, cols t*160 + o*10 + c  (co-order is (o, c))
    wl = np.ascontiguousarray(Wf.transpose(0, 3, 2, 1)).reshape(KT, 128, CO)
    wl = np.ascontiguousarray(wl.transpose(1, 0, 2)).reshape(128, KT * CO).astype(bf)
    s8x = np.zeros((128, 16), np.float32)
    s8x[np.arange(128), np.arange(128) // 8] = 1.0
    e2 = np.zeros((64, 128), np.float32)
    e2[np.arange(128) % 64, np.arange(128)] = 1.0
    cpb[:, 256:272] = s8x.astype(bf)
    cpb[0:16, 272:400] = s8x.T.astype(bf)
    cpb[0:64, 400:528] = e2.astype(bf)
    cpf[0:16, 4:5] = 1.0
    cpf[0:1, 5:69] = 1.0

    shared = {"cpb": cpb, "cpf": cpf, "w2s": w2, "wlb": wl}
    in_maps = []
    for c in range(NCORES):
        xs = np.zeros(BL * 784 + 8, bf)
        xs[:BL * 784] = xf[c * BL:(c + 1) * BL].reshape(-1).astype(bf)
        in_maps.append({"xin": xs, **shared})
    return in_maps


def kernel(x, conv1_w, conv1_b, conv2_w, conv2_b, W):
    if "nc" not in _CACHE:
        _CACHE["nc"] = build_nc()
    nc = _CACHE["nc"]
    in_maps = make_in_maps(x, conv1_w, conv1_b, conv2_w, conv2_b, W)
    res = run_bass_kernel_spmd(nc, in_maps, list(range(NCORES)), trace=False)
    outs = [res.results[c]["out"] for c in range(NCORES)]
    full = np.concatenate(outs, axis=0).reshape(B, O, C)
    return np.ascontiguousarray(full.transpose(0, 2, 1)).astype(np.float32)
